# revision 63
# baseline (speedup 1.0000x reference)
"""Trainium2 Bass kernel for causal multi-head attention with RoPE.

Problem: x[2,2048,2048] -> qkv proj -> RoPE(q,k) -> causal softmax attention
(16 heads, hd=128) -> out proj.  Sharding: tensor-parallel over heads
(2 heads/core x 8 cores); the output projection contraction is restored
with one AllToAll per batch (head-shards -> sequence-shards), overlapped
with the other batch's compute, so each core computes a disjoint
[2, 256, 2048] slice of the final output.

All matmul operands are bf16 (PSUM accumulation stays fp32): halves
LDWEIGHTS time (the fp32 weight-load was the PE cadence limiter), halves
HBM/DMA traffic and the A2A payload, and doubles DVE throughput for the
elementwise work.  Softmax skips the max-subtraction (scores are O(1) by
construction); the causal mask is accumulated into PSUM as a -1e9
constant via a PE identity-matmul; softmax denominators are
partition-reduced and broadcast back with tiny ones-matmuls on the PE.
"""

import os
import sys

if "/opt/trn_rl_repo" not in sys.path:
    sys.path.insert(0, "/opt/trn_rl_repo")

import numpy as np
import ml_dtypes

B, S, D = 2, 2048, 2048
H, HD = 16, 128
NCORES = 8
HPC = H // NCORES          # heads per core (2)
ROPE_BASE = 10000.0
SCALE = 1.0 / float(np.sqrt(HD))
SC = 512                   # QKV matmul free-dim chunk (s positions)
KSUB = D // 128            # 16 contraction subtiles
SCW = S // NCORES          # 256: per-core output cols per batch

_CACHE = {}


def _install_trace_shim():
    """Optionally register the axon NTFF profile hook (for test.py tracing)."""
    try:
        import types

        if "antenv.axon_hooks" in sys.modules:
            return True
        import antenv
        from trn_agent_boot.trn_boot import _ntff_profile_via_ctypes

        hook = _ntff_profile_via_ctypes("/opt/axon/libaxon_pjrt.so")
        mod = types.ModuleType("antenv.axon_hooks")
        _state = {"hook": hook}
        mod.get_axon_ntff_profile_hook = lambda: _state["hook"]
        mod.set_axon_ntff_profile_hook = lambda h: _state.__setitem__("hook", h)
        sys.modules["antenv.axon_hooks"] = mod
        antenv.axon_hooks = mod
        return True
    except Exception:
        return False


def _build():
    import concourse.bass as bass  # noqa: F401
    import concourse.bass_isa as bass_isa
    import concourse.mybir as mybir
    import concourse.tile as tile
    from concourse import bacc
    from concourse.masks import make_identity

    f32 = mybir.dt.float32
    f32r = mybir.dt.float32r
    bf = mybir.dt.bfloat16
    EXP = mybir.ActivationFunctionType.Exp

    nc = bacc.Bacc("TRN2", target_bir_lowering=False, debug=False,
                   num_devices=NCORES)

    # chunk-major DRAM layouts: each chunk/tile is one contiguous 4-16KB
    # line per partition, so a single DMA runs at full ring bandwidth.
    xT = nc.dram_tensor("xT", [128, B * S // SC, KSUB, SC], bf,
                        kind="ExternalInput")
    wqkv = nc.dram_tensor("wqkv", [128, 2 * HPC, KSUB, 128], bf,
                          kind="ExternalInput")
    wvg = nc.dram_tensor("wvg", [128, KSUB, HPC * HD], bf,
                         kind="ExternalInput")
    wout = nc.dram_tensor("wout", [128, 4, KSUB, 512], bf,
                          kind="ExternalInput")
    cosg = nc.dram_tensor("cosg", [128, S], bf, kind="ExternalInput")
    sing = nc.dram_tensor("sing", [128, S], bf, kind="ExternalInput")
    mneg = nc.dram_tensor("mneg", [128, 128], bf, kind="ExternalInput")
    y = nc.dram_tensor("y", [B, SCW, D], f32, kind="ExternalOutput")

    NQC = S // SC          # qkv s-chunks per batch
    NKT = S // 128         # 16 key tiles
    VOFF = 2 * HPC * HD    # v block column offset in w_sb (512)

    with tile.TileContext(nc) as tc:
        with tc.tile_pool(name="const", bufs=1) as cp, \
             tc.tile_pool(name="stage", bufs=1) as stp, \
             tc.tile_pool(name="dram", bufs=1, space="DRAM") as dp, \
             tc.tile_pool(name="psA", bufs=2, space="PSUM") as psA, \
             tc.tile_pool(name="psOut", bufs=1, space="PSUM") as psO, \
             tc.tile_pool(name="w", bufs=1) as wp, \
             tc.tile_pool(name="xc", bufs=2) as xp, \
             tc.tile_pool(name="wo2", bufs=1) as wop, \
             tc.tile_pool(name="lhs0", bufs=1) as lp, \
             tc.tile_pool(name="qkv", bufs=1) as qp, \
             tc.tile_pool(name="attn", bufs=1) as ap_, \
             tc.tile_pool(name="rotp", bufs=1) as rp, \
             tc.tile_pool(name="small", bufs=4) as ep:

            cos_sb = cp.tile([128, S], bf, name="cos_sb")
            sin_sb = cp.tile([128, S], bf, name="sin_sb")
            mneg_sb = cp.tile([128, 128], bf, name="mneg_sb")
            ident = cp.tile([128, 128], f32, name="ident")
            identB = cp.tile([128, 128], bf, name="identB")
            onescB = cp.tile([128, 1], bf, name="onescB")
            onesr = cp.tile([1, 128], f32, name="onesr")
            onesrR = cp.tile([1, 128], f32r, name="onesrR")

            # startup loads spread across the 3 DMA-capable queues
            # (sync/scalar/gpsimd) so the first chunk's matmuls are never
            # single-queue bound.
            # The 16 DMA engines are one shared ~400GB/s pool, so what
            # matters is byte ORDER across queues: first weight tiles, then
            # all of chunk 0 split 3 ways, then the rest.
            engs = [nc.sync, nc.scalar, nc.gpsimd]
            wqk_t = []
            for m in range(3):
                wt = wp.tile([128, KSUB, 128], bf, tag=f"w{m}", name=f"w{m}")
                engs[m].dma_start(wt[:], wqkv.ap()[:, m])
                wqk_t.append(wt)
            xc0 = xp.tile([128, KSUB, SC], bf, tag="xc", name="xc")
            for e, (k0, k1) in enumerate(((0, 6), (6, 11), (11, 16))):
                engs[e].dma_start(xc0[:, k0:k1], xT.ap()[:, 0, k0:k1])
            wt = wp.tile([128, KSUB, 128], bf, tag="w3", name="w3")
            nc.sync.dma_start(wt[:], wqkv.ap()[:, 3])
            wqk_t.append(wt)
            wv_t = wp.tile([128, KSUB, HPC * HD], bf, tag="wv", name="wv")
            nc.gpsimd.dma_start(wv_t[:], wvg.ap())

            make_identity(nc, ident[:])
            nc.vector.tensor_copy(identB[:], ident[:])
            nc.vector.memset(onescB[:], 1.0)
            nc.vector.memset(onesr[:], 1.0)
            nc.vector.tensor_copy(onesrR[:], onesr[:])

            ibs = {(b, h): dp.tile([NCORES, 128, SCW], bf, name=f"ib{b}{h}")
                   for b in range(B) for h in range(HPC)}
            obs = {(b, h): dp.tile([NCORES, 128, SCW], bf, name=f"ob{b}{h}")
                   for b in range(B) for h in range(HPC)}

            def qkv_rope(b, pre_xc=None, mid_hook=None):
                qkT = qp.tile([128, 2 * HPC, S], bf, tag="qkT")
                Vn = qp.tile([128, NKT, HPC * HD], bf, tag="Vn")

                # RoPE, fused halves (sin grid stored pre-swapped):
                # rt[0:64] = t[64:128]*(-sin); rt[64:128] = t[0:64]*(+sin);
                # t *= cos; t += rt.  Emitted per-m inside the last chunk so
                # the vector engine isn't backlogged when attention starts.
                def rope(m):
                    rt = rp.tile([128, S], bf, tag="rot", name="rt")
                    nc.vector.tensor_mul(rt[0:64, :],
                                         qkT[64:128, m],
                                         sin_sb[64:128, :])
                    nc.vector.tensor_mul(rt[64:128, :],
                                         qkT[0:64, m],
                                         sin_sb[0:64, :])
                    nc.vector.tensor_mul(qkT[:, m], qkT[:, m], cos_sb[:])
                    nc.vector.tensor_add(qkT[:, m], qkT[:, m], rt[:])

                for sc in range(NQC):
                    if sc == 0 and pre_xc is not None:
                        xc = pre_xc
                    else:
                        xc = xp.tile([128, KSUB, SC], bf, tag="xc", name="xc")
                        eng = (nc.scalar if sc == 1 else
                               nc.gpsimd if sc == 3 else nc.sync) \
                            if b == 0 else nc.sync
                        eng.dma_start(xc[:], xT.ap()[:, b * NQC + sc])
                    if sc == NQC - 1 and mid_hook is not None:
                        mid_hook()
                    # two q/k chains share one 2-bank PSUM tile -> one
                    # [128,1024] copy each; all four V chains share one tile
                    for mp in range(HPC):
                        ps = psA.tile([128, 1024], f32, tag="bank")
                        for mm in range(2):
                            m = 2 * mp + mm
                            for k in range(KSUB):
                                nc.tensor.matmul(
                                    ps[:, mm * SC:(mm + 1) * SC],
                                    wqk_t[m][:, k],
                                    xc[:, k],
                                    start=(k == 0), stop=(k == KSUB - 1))
                        nc.vector.tensor_copy(
                            qkT[:, 2 * mp:2 * mp + 2,
                                sc * SC:(sc + 1) * SC], ps[:])
                        if sc == NQC - 1:
                            rope(2 * mp)
                            rope(2 * mp + 1)
                    ps = psA.tile([128, 1024], f32, tag="bank")
                    for st2 in range(SC // 128):
                        for k in range(KSUB):
                            nc.tensor.matmul(
                                ps[:, st2 * HPC * HD:(st2 + 1) * HPC * HD],
                                xc[:, k, st2 * 128:(st2 + 1) * 128],
                                wv_t[:, k],
                                start=(k == 0), stop=(k == KSUB - 1))
                    nc.vector.tensor_copy(
                        Vn[:, sc * (SC // 128):(sc + 1) * (SC // 128)],
                        ps[:])

                return qkT, Vn

            def attention(b, h, qkT, Vn, fillers=()):
                fillers = list(fillers)
                outT = psO.tile([128, S], f32, tag="outT")
                acc = ap_.tile([128, S], bf, tag="acc")

                def emit_av(kt, off, ets, nch):
                    q0 = 512 * (kt // 4)
                    for c in range(nch):
                        qs = q0 + c * 512
                        o = off if c == 0 else 0
                        b0 = (c % 2) * 512
                        nc.tensor.matmul(
                            outT[:, qs + o:qs + 512],
                            Vn[:, kt, h * 128:(h + 1) * 128],
                            ets[c // 2][:, b0 + o:b0 + 512],
                            start=(kt == 0),
                            stop=(kt == 4 * (qs // 512) + 3))

                st = rp.tile([128, S], bf, tag="rot", name="st")

                srows = {}

                def finalize_a(j):
                    # denominator: partition-reduce via ones-matmul, then
                    # reciprocal on vector.  Emitted two kts before
                    # finalize_b so the PE's bp matmul never waits on the
                    # vector queue.
                    rps = psA.tile([128, 1024], f32, tag="bank")
                    nc.tensor.matmul(rps[0:1, :512], onescB[:],
                                     acc[:, j * 512:(j + 1) * 512],
                                     start=True, stop=True)
                    srow = stp.tile([1, 512], f32, tag="srow")
                    srowR = stp.tile([1, 512], f32r, tag="srowR")
                    nc.vector.reciprocal_approx_fast(srow[:], rps[0:1, :512])
                    nc.vector.tensor_copy(srowR[:], srow[:])
                    srows[j] = srowR

                def finalize_b(j):
                    # K=1 broadcast matmul, normalize, ship.
                    bp = psA.tile([128, 1024], f32, tag="bank")
                    nc.tensor.matmul(bp[:, :512], onesrR[:], srows[j][:],
                                     start=True, stop=True)
                    sl = slice(j * 512, (j + 1) * 512)
                    nc.vector.tensor_copy(st[:, sl], outT[:, sl])
                    nc.vector.tensor_mul(st[:, sl], st[:, sl], bp[:, :512])
                    for jj in (2 * j, 2 * j + 1):
                        nc.sync.dma_start(ibs[(b, h)][jj],
                                          st[:, jj * SCW:(jj + 1) * SCW])

                prev = None
                for kt in range(NKT):
                    q0 = 512 * (kt // 4)
                    off = 128 * (kt % 4)   # causal start within chunk 0
                    nch = (S - q0) // 512
                    npair = (nch + 1) // 2
                    pts = [psA.tile([128, 1024], f32, tag="bank", name="pt")
                           for _ in range(npair)]
                    # -1e9 upper-tri mask for the diagonal 128 block only;
                    # the rest of chunk 0 is a separate accumulation group.
                    nc.tensor.matmul(pts[0][:, off:off + 128], identB[:],
                                     mneg_sb[:], start=True, stop=False)
                    kT = qkT[:, HPC + h, kt * 128:(kt + 1) * 128]
                    nc.tensor.matmul(
                        pts[0][:, off:off + 128], kT,
                        qkT[:, h, q0 + off:q0 + off + 128],
                        start=False, stop=True)
                    if off + 128 < 512:
                        nc.tensor.matmul(
                            pts[0][:, off + 128:512], kT,
                            qkT[:, h, q0 + off + 128:q0 + 512],
                            start=True, stop=True)
                    for c in range(1, nch):
                        qs = q0 + c * 512
                        b0 = (c % 2) * 512
                        nc.tensor.matmul(
                            pts[c // 2][:, b0:b0 + 512], kT,
                            qkT[:, h, qs:qs + 512],
                            start=True, stop=True)
                    if prev is not None:
                        emit_av(*prev)
                    if kt >= 5 and (kt - 5) % 4 == 0 and kt < 17:
                        finalize_a((kt - 5) // 4)
                    if kt >= 7 and (kt - 7) % 4 == 0:
                        finalize_b((kt - 7) // 4)
                    ets = []
                    for p in range(npair):
                        w = min(1024, (nch - 2 * p) * 512)
                        o2 = off if p == 0 else 0
                        et = ep.tile([128, 1024], bf, tag="expT", bufs=3)
                        ets.append(et)
                        nc.scalar.activation(et[:, o2:w], pts[p][:, o2:w],
                                             EXP, scale=SCALE)
                    for p in range(npair):
                        w = min(1024, (nch - 2 * p) * 512)
                        o2 = off if p == 0 else 0
                        base = q0 + p * 1024
                        if kt == 0:
                            nc.vector.tensor_copy(acc[:, base:base + w],
                                                  ets[p][:, :w])
                        else:
                            nc.vector.tensor_add(acc[:, base + o2:base + w],
                                                 acc[:, base + o2:base + w],
                                                 ets[p][:, o2:w])
                    if fillers and kt >= 7:
                        fillers.pop(0)()
                    prev = (kt, off, ets, nch)
                emit_av(*prev)
                finalize_a(3)
                finalize_b(3)
                while fillers:
                    fillers.pop(0)()

            def load_lhs(b, pool, tag, eng):
                # k-subtile order hh*8+i <-> global head 2i+hh (wout is
                # permuted host-side to match); one queue per head-pair
                lhs = pool.tile([128, KSUB, SCW], bf, tag=tag,
                                name=f"lhs{b}")
                for hh in range(HPC):
                    eng[hh % len(eng)].dma_start(
                        lhs[:, hh * NCORES:(hh + 1) * NCORES, :],
                        obs[(b, hh)][:].rearrange("i p s -> p i s"))
                return lhs

            def outproj(b, lhs, wos):
                # two n-chunks (same output rows) share one PSUM tile ->
                # one [128,1024] copy + one y DMA per pair
                for m in range(SCW // 128):
                    for np_ in range(2):
                        ps = psA.tile([128, 1024], f32, tag="bank")
                        for nn in range(2):
                            wo = wos[2 * np_ + nn]
                            for k in range(KSUB):
                                nc.tensor.matmul(
                                    ps[:, nn * 512:(nn + 1) * 512],
                                    lhs[:, k, m * 128:(m + 1) * 128],
                                    wo[:, k],
                                    start=(k == 0), stop=(k == KSUB - 1))
                        ys = ep.tile([128, 1024], f32, tag="ysT", name="ys",
                                     bufs=2)
                        nc.vector.tensor_copy(ys[:], ps[:])
                        nc.sync.dma_start(
                            y.ap()[b, m * 128:(m + 1) * 128,
                                   np_ * 1024:(np_ + 1) * 1024],
                            ys[:])

            def a2a(b, h):
                nc.gpsimd.collective_compute(
                    "AllToAll", mybir.AluOpType.bypass,
                    replica_groups=[list(range(NCORES))],
                    ins=[ibs[(b, h)].opt()], outs=[obs[(b, h)].opt()])

            # batch 0 compute; its A2A runs while batch 1 computes; all of
            # batch-0's outproj runs before batch-1's so the last A2A hides
            # under batch-0 outproj matmuls.
            wos = {}

            def prefetch_wo(n, pool, tag, eng):
                wo = pool.tile([128, KSUB, 512], bf, tag=tag, name=f"wo{n}")
                eng.dma_start(wo[:], wout.ap()[:, n])
                wos[n] = wo

            def load_consts():
                # rope/mask constants aren't needed until ~100us in;
                # emitting them after the chunk loop keeps the scalar queue
                # clear for x chunk slices during startup.
                nc.scalar.dma_start(cos_sb[:], cosg.ap())
                nc.scalar.dma_start(sin_sb[:], sing.ap())
                nc.scalar.dma_start(mneg_sb[:], mneg.ap())

            with nc.named_scope("qkv0"):
                qkT, Vn = qkv_rope(0, pre_xc=xc0, mid_hook=load_consts)
            with nc.named_scope("attn00"):
                attention(0, 0, qkT, Vn)
            a2a(0, 0)
            # prefetch batch-1 chunk-0 x while attention(0,1) computes
            xc1 = xp.tile([128, KSUB, SC], bf, tag="xc", name="xc")
            nc.scalar.dma_start(xc1[:], xT.ap()[:, NQC])
            # w_out chunks 2,3 live in their own pool; fetch early
            prefetch_wo(2, wop, "woA", nc.scalar)
            prefetch_wo(3, wop, "woB", nc.scalar)
            with nc.named_scope("attn01"):
                attention(0, 1, qkT, Vn)
            a2a(0, 1)
            with nc.named_scope("qkv1"):
                qkT, Vn = qkv_rope(1, pre_xc=xc1)
            # w_out chunks 0,1 reuse the (now idle) xc slots; gpsimd's DMA
            # queue is free after startup, and using it keeps the sync
            # queue clear for the attention ibs shipments.  lhs0 first: the
            # first outproj matmul needs it before the wo chunks.
            lhs0 = load_lhs(0, lp, "lhs0", [nc.gpsimd, nc.scalar])
            prefetch_wo(0, xp, "xc", nc.gpsimd)
            prefetch_wo(1, xp, "xc", nc.gpsimd)
            with nc.named_scope("attn10"):
                attention(1, 0, qkT, Vn)
            a2a(1, 0)
            with nc.named_scope("attn11"):
                attention(1, 1, qkT, Vn)
            a2a(1, 1)
            lhs1 = load_lhs(1, qp, "Vn", [nc.scalar, nc.gpsimd])
            with nc.named_scope("oproj0"):
                outproj(0, lhs0, wos)
            with nc.named_scope("oproj1"):
                outproj(1, lhs1, wos)

    nc.finalize()
    return nc


def _host_inputs(x, w_qkv, w_out):
    bf = ml_dtypes.bfloat16
    # x^T, chunk-major: [128, n_chunks, KSUB, SC] so one chunk is a single
    # contiguous 16KB line per partition.
    xTr = np.ascontiguousarray(
        x.reshape(B * S, D).T.reshape(KSUB, 128, B * S)
        .transpose(1, 0, 2).reshape(128, KSUB, B * S // SC, SC)
        .transpose(0, 2, 1, 3)
    ).astype(bf)
    horder = [2 * i + hh for hh in range(HPC) for i in range(NCORES)]
    woutr = np.ascontiguousarray(
        w_out.reshape(H, HD, D)[horder].transpose(1, 0, 2)
        .reshape(128, KSUB, 4, 512).transpose(0, 2, 1, 3)).astype(bf)

    half = HD // 2
    inv = (1.0 / (ROPE_BASE ** (np.arange(half, dtype=np.float32) / half))
           ).astype(np.float32)
    ang = (np.arange(S, dtype=np.float32)[:, None] * inv[None, :])  # [S, 64]
    c = np.cos(ang).astype(np.float32).T      # [64, S]
    s = np.sin(ang).astype(np.float32).T
    cosg = np.ascontiguousarray(np.concatenate([c, c], axis=0)).astype(bf)
    # pre-swapped: rows 0:64 = +sin (consumed against t[0:64] -> rt[64:128]),
    # rows 64:128 = -sin (consumed against t[64:128] -> rt[0:64])
    sing = np.ascontiguousarray(np.concatenate([s, -s], axis=0)).astype(bf)

    # mneg[p, j] = 0 where j >= p else -1e9 (upper-tri of the diagonal
    # 128-block).
    u = np.arange(128)[None, :]
    p = np.arange(128)[:, None]
    mneg = np.where(u >= p, 0.0, -1e9).astype(bf)

    maps = []
    for i in range(NCORES):
        h0, h1 = 2 * i, 2 * i + 1
        blocks = []
        for base in (0, D, 2 * D):
            blocks.append(w_qkv[:, base + 128 * h0:base + 128 * (h0 + 1)])
            blocks.append(w_qkv[:, base + 128 * h1:base + 128 * (h1 + 1)])
        shard = np.concatenate(blocks, axis=1)  # [D, 768]
        shard = shard.reshape(KSUB, 128, 3 * HPC * HD).transpose(1, 0, 2)
        # q/k tiles tile-major [128, 4, KSUB, 128]; v separate [128,KSUB,256]
        qk = np.ascontiguousarray(
            shard[:, :, :512].reshape(128, KSUB, 4, 128)
            .transpose(0, 2, 1, 3)).astype(bf)
        wv = np.ascontiguousarray(shard[:, :, 512:]).astype(bf)
        maps.append({"xT": xTr, "wqkv": qk, "wvg": wv, "wout": woutr,
                     "cosg": cosg, "sing": sing, "mneg": mneg})
    return maps


def kernel(x, w_qkv, w_out):
    from concourse.bass_utils import run_bass_kernel_spmd

    x = np.asarray(x, dtype=np.float32)
    w_qkv = np.asarray(w_qkv, dtype=np.float32)
    w_out = np.asarray(w_out, dtype=np.float32)

    if "nc" not in _CACHE:
        _CACHE["nc"] = _build()
    nc = _CACHE["nc"]

    trace = bool(int(os.environ.get("KERNEL_TRACE", "0")))
    if trace:
        trace = _install_trace_shim()

    in_maps = _host_inputs(x, w_qkv, w_out)
    res = run_bass_kernel_spmd(nc, in_maps, core_ids=list(range(NCORES)),
                               trace=trace)
    _CACHE["last_result"] = res
    # y per core i: [B, 256, D] = output rows [b*2048 + i*256, +256)
    full = np.empty((B * S, D), dtype=np.float32)
    for i in range(NCORES):
        yi = res.results[i]["y"]
        for b in range(B):
            full[b * S + i * SCW: b * S + (i + 1) * SCW] = yi[b]
    return full.reshape(B, S, D)


# revision 64
# speedup vs baseline: 1.1060x; 1.1060x over previous
"""Trainium2 Bass kernel for causal multi-head attention with RoPE.

Problem: x[2,2048,2048] -> qkv proj -> RoPE(q,k) -> causal softmax attention
(16 heads, hd=128) -> out proj.  Sharding: tensor-parallel over heads
(2 heads/core x 8 cores); the output projection contraction is restored
with one AllToAll per batch (head-shards -> sequence-shards), overlapped
with the other batch's compute, so each core computes a disjoint
[2, 256, 2048] slice of the final output.

All matmul operands are bf16 (PSUM accumulation stays fp32): halves
LDWEIGHTS time (the fp32 weight-load was the PE cadence limiter), halves
HBM/DMA traffic and the A2A payload, and doubles DVE throughput for the
elementwise work.  Softmax skips the max-subtraction (scores are O(1) by
construction); the causal mask is accumulated into PSUM as a -1e9
constant via a PE identity-matmul; softmax denominators are
partition-reduced and broadcast back with tiny ones-matmuls on the PE.
"""

import os
import sys

if "/opt/trn_rl_repo" not in sys.path:
    sys.path.insert(0, "/opt/trn_rl_repo")

import numpy as np
import ml_dtypes

B, S, D = 2, 2048, 2048
H, HD = 16, 128
NCORES = 8
HPC = H // NCORES          # heads per core (2)
ROPE_BASE = 10000.0
SCALE = 1.0 / float(np.sqrt(HD))
SC = 512                   # QKV matmul free-dim chunk (s positions)
KSUB = D // 128            # 16 contraction subtiles
SCW = S // NCORES          # 256: per-core output cols per batch

_CACHE = {}


def _install_trace_shim():
    """Optionally register the axon NTFF profile hook (for test.py tracing)."""
    try:
        import types

        if "antenv.axon_hooks" in sys.modules:
            return True
        import antenv
        from trn_agent_boot.trn_boot import _ntff_profile_via_ctypes

        hook = _ntff_profile_via_ctypes("/opt/axon/libaxon_pjrt.so")
        mod = types.ModuleType("antenv.axon_hooks")
        _state = {"hook": hook}
        mod.get_axon_ntff_profile_hook = lambda: _state["hook"]
        mod.set_axon_ntff_profile_hook = lambda h: _state.__setitem__("hook", h)
        sys.modules["antenv.axon_hooks"] = mod
        antenv.axon_hooks = mod
        return True
    except Exception:
        return False


def _build():
    import concourse.bass as bass  # noqa: F401
    import concourse.bass_isa as bass_isa
    import concourse.mybir as mybir
    import concourse.tile as tile
    from concourse import bacc
    from concourse.masks import make_identity

    f32 = mybir.dt.float32
    f32r = mybir.dt.float32r
    bf = mybir.dt.bfloat16
    EXP = mybir.ActivationFunctionType.Exp

    nc = bacc.Bacc("TRN2", target_bir_lowering=False, debug=False,
                   num_devices=NCORES)

    # chunk-major DRAM layouts: each chunk/tile is one contiguous 4-16KB
    # line per partition, so a single DMA runs at full ring bandwidth.
    xT = nc.dram_tensor("xT", [128, B * S // SC, KSUB, SC], bf,
                        kind="ExternalInput")
    wqkv = nc.dram_tensor("wqkv", [128, 2 * HPC, KSUB, 128], bf,
                          kind="ExternalInput")
    wvg = nc.dram_tensor("wvg", [128, KSUB, HPC * HD], bf,
                         kind="ExternalInput")
    wout = nc.dram_tensor("wout", [128, 4, KSUB, 512], bf,
                          kind="ExternalInput")
    cosg = nc.dram_tensor("cosg", [128, S], bf, kind="ExternalInput")
    sing = nc.dram_tensor("sing", [128, S], bf, kind="ExternalInput")
    mneg = nc.dram_tensor("mneg", [128, 128], bf, kind="ExternalInput")
    y = nc.dram_tensor("y", [B, SCW, D], f32, kind="ExternalOutput")

    NQC = S // SC          # qkv s-chunks per batch
    NKT = S // 128         # 16 key tiles
    VOFF = 2 * HPC * HD    # v block column offset in w_sb (512)

    with tile.TileContext(nc) as tc:
        with tc.tile_pool(name="const", bufs=1) as cp, \
             tc.tile_pool(name="stage", bufs=1) as stp, \
             tc.tile_pool(name="dram", bufs=1, space="DRAM") as dp, \
             tc.tile_pool(name="psA", bufs=4, space="PSUM") as psA, \
             tc.tile_pool(name="psOut", bufs=1, space="PSUM") as psO, \
             tc.tile_pool(name="w", bufs=1) as wp, \
             tc.tile_pool(name="xc", bufs=2) as xp, \
             tc.tile_pool(name="wo2", bufs=1) as wop, \
             tc.tile_pool(name="lhs0", bufs=1) as lp, \
             tc.tile_pool(name="qkv", bufs=1) as qp, \
             tc.tile_pool(name="attn", bufs=1) as ap_, \
             tc.tile_pool(name="rotp", bufs=1) as rp, \
             tc.tile_pool(name="small", bufs=4) as ep:

            cos_sb = cp.tile([128, S], bf, name="cos_sb")
            sin_sb = cp.tile([128, S], bf, name="sin_sb")
            mneg_sb = cp.tile([128, 128], bf, name="mneg_sb")
            ident = cp.tile([128, 128], f32, name="ident")
            identB = cp.tile([128, 128], bf, name="identB")
            onescB = cp.tile([128, 1], bf, name="onescB")
            onesr = cp.tile([1, 128], f32, name="onesr")
            onesrR = cp.tile([1, 128], f32r, name="onesrR")

            # startup loads spread across the 3 DMA-capable queues
            # (sync/scalar/gpsimd) so the first chunk's matmuls are never
            # single-queue bound.
            # The 16 DMA engines are one shared ~400GB/s pool, so what
            # matters is byte ORDER across queues: first weight tiles, then
            # all of chunk 0 split 3 ways, then the rest.
            engs = [nc.sync, nc.scalar, nc.gpsimd]
            wqk_t = []
            for m in range(3):
                wt = wp.tile([128, KSUB, 128], bf, tag=f"w{m}", name=f"w{m}")
                engs[m].dma_start(wt[:], wqkv.ap()[:, m])
                wqk_t.append(wt)
            xc0 = xp.tile([128, KSUB, SC], bf, tag="xc", name="xc")
            for e, (k0, k1) in enumerate(((0, 6), (6, 11), (11, 16))):
                engs[e].dma_start(xc0[:, k0:k1], xT.ap()[:, 0, k0:k1])
            wt = wp.tile([128, KSUB, 128], bf, tag="w3", name="w3")
            nc.sync.dma_start(wt[:], wqkv.ap()[:, 3])
            wqk_t.append(wt)
            wv_t = wp.tile([128, KSUB, HPC * HD], bf, tag="wv", name="wv")
            nc.gpsimd.dma_start(wv_t[:], wvg.ap())

            make_identity(nc, ident[:])
            nc.vector.tensor_copy(identB[:], ident[:])
            nc.vector.memset(onescB[:], 1.0)
            nc.vector.memset(onesr[:], 1.0)
            nc.vector.tensor_copy(onesrR[:], onesr[:])

            ibs = {(b, h): dp.tile([NCORES, 128, SCW], bf, name=f"ib{b}{h}")
                   for b in range(B) for h in range(HPC)}
            obs = {(b, h): dp.tile([NCORES, 128, SCW], bf, name=f"ob{b}{h}")
                   for b in range(B) for h in range(HPC)}

            def qkv_rope(b, pre_xc=None, mid_hook=None):
                qkT = qp.tile([128, 2 * HPC, S], bf, tag="qkT")
                Vn = qp.tile([128, NKT, HPC * HD], bf, tag="Vn")

                # RoPE, fused halves (sin grid stored pre-swapped):
                # rt[0:64] = t[64:128]*(-sin); rt[64:128] = t[0:64]*(+sin);
                # t *= cos; t += rt.  Emitted per-m inside the last chunk so
                # the vector engine isn't backlogged when attention starts.
                def rope(m):
                    rt = rp.tile([128, S], bf, tag="rot", name="rt")
                    nc.vector.tensor_mul(rt[0:64, :],
                                         qkT[64:128, m],
                                         sin_sb[64:128, :])
                    nc.vector.tensor_mul(rt[64:128, :],
                                         qkT[0:64, m],
                                         sin_sb[0:64, :])
                    nc.vector.tensor_mul(qkT[:, m], qkT[:, m], cos_sb[:])
                    nc.vector.tensor_add(qkT[:, m], qkT[:, m], rt[:])

                for sc in range(NQC):
                    if sc == 0 and pre_xc is not None:
                        xc = pre_xc
                    else:
                        xc = xp.tile([128, KSUB, SC], bf, tag="xc", name="xc")
                        eng = (nc.scalar if sc == 1 else
                               nc.gpsimd if sc == 3 else nc.sync) \
                            if b == 0 else nc.sync
                        eng.dma_start(xc[:], xT.ap()[:, b * NQC + sc])
                    if sc == NQC - 1 and mid_hook is not None:
                        mid_hook()
                    for m in range(2 * HPC):
                        ps = psA.tile([128, 512], f32, tag="bank")
                        for k in range(KSUB):
                            nc.tensor.matmul(
                                ps[:, :SC],
                                wqk_t[m][:, k],
                                xc[:, k],
                                start=(k == 0), stop=(k == KSUB - 1))
                        nc.vector.tensor_copy(
                            qkT[:, m, sc * SC:(sc + 1) * SC], ps[:, :SC])
                        if sc == NQC - 1:
                            rope(m)
                    for st2 in range(SC // 128):
                        ps = psA.tile([128, 512], f32, tag="bank")
                        for k in range(KSUB):
                            nc.tensor.matmul(
                                ps[:, :HPC * HD],
                                xc[:, k, st2 * 128:(st2 + 1) * 128],
                                wv_t[:, k],
                                start=(k == 0), stop=(k == KSUB - 1))
                        nc.vector.tensor_copy(
                            Vn[:, sc * (SC // 128) + st2], ps[:, :HPC * HD])

                return qkT, Vn

            def attention(b, h, qkT, Vn, fillers=()):
                fillers = list(fillers)
                outT = psO.tile([128, S], f32, tag="outT")
                acc = ap_.tile([128, S], bf, tag="acc")

                def emit_av(kt, off, ets):
                    q0 = 512 * (kt // 4)
                    for c in range(len(ets)):
                        qs = q0 + c * 512
                        o = off if c == 0 else 0
                        nc.tensor.matmul(
                            outT[:, qs + o:qs + 512],
                            Vn[:, kt, h * 128:(h + 1) * 128],
                            ets[c][:, o:512],
                            start=(kt == 0),
                            stop=(kt == 4 * (qs // 512) + 3))

                st = rp.tile([128, S], bf, tag="rot", name="st")

                srows = {}

                def finalize_a(j):
                    # denominator: partition-reduce via ones-matmul, then
                    # reciprocal on vector.  Emitted one kt before
                    # finalize_b so the PE's bp matmul never waits on the
                    # vector queue.
                    rps = psA.tile([128, 512], f32, tag="bank")
                    nc.tensor.matmul(rps[0:1, :], onescB[:],
                                     acc[:, j * 512:(j + 1) * 512],
                                     start=True, stop=True)
                    srow = stp.tile([1, 512], f32, tag="srow")
                    srowR = stp.tile([1, 512], f32r, tag="srowR")
                    nc.vector.reciprocal_approx_fast(srow[:], rps[0:1, :])
                    nc.vector.tensor_copy(srowR[:], srow[:])
                    srows[j] = srowR

                def finalize_b(j):
                    # K=1 broadcast matmul, normalize, ship.
                    bp = psA.tile([128, 512], f32, tag="bank")
                    nc.tensor.matmul(bp[:], onesrR[:], srows[j][:],
                                     start=True, stop=True)
                    sl = slice(j * 512, (j + 1) * 512)
                    nc.vector.tensor_copy(st[:, sl], outT[:, sl])
                    nc.vector.tensor_mul(st[:, sl], st[:, sl], bp[:])
                    for jj in (2 * j, 2 * j + 1):
                        nc.sync.dma_start(ibs[(b, h)][jj],
                                          st[:, jj * SCW:(jj + 1) * SCW])

                prev = None
                for kt in range(NKT):
                    q0 = 512 * (kt // 4)
                    off = 128 * (kt % 4)   # causal start within chunk 0
                    nch = (S - q0) // 512
                    sps = []
                    for c in range(nch):
                        sp = psA.tile([128, 512], f32, tag="bank")
                        sps.append(sp)
                    # -1e9 upper-tri mask for the diagonal 128 block only;
                    # the rest of chunk 0 is a separate accumulation group.
                    nc.tensor.matmul(sps[0][:, off:off + 128], identB[:],
                                     mneg_sb[:], start=True, stop=False)
                    kT = qkT[:, HPC + h, kt * 128:(kt + 1) * 128]
                    nc.tensor.matmul(
                        sps[0][:, off:off + 128], kT,
                        qkT[:, h, q0 + off:q0 + off + 128],
                        start=False, stop=True)
                    if off + 128 < 512:
                        nc.tensor.matmul(
                            sps[0][:, off + 128:512], kT,
                            qkT[:, h, q0 + off + 128:q0 + 512],
                            start=True, stop=True)
                    for c in range(1, nch):
                        qs = q0 + c * 512
                        nc.tensor.matmul(
                            sps[c][:], kT,
                            qkT[:, h, qs:qs + 512],
                            start=True, stop=True)
                    if prev is not None:
                        emit_av(*prev)
                    if kt >= 5 and (kt - 5) % 4 == 0 and kt < 17:
                        finalize_a((kt - 5) // 4)
                    if kt >= 7 and (kt - 7) % 4 == 0:
                        finalize_b((kt - 7) // 4)
                    ets = []
                    for c in range(nch):
                        o = off if c == 0 else 0
                        et = ep.tile([128, 512], bf, tag="expT", bufs=6)
                        ets.append(et)
                        nc.scalar.activation(et[:, o:512], sps[c][:, o:512],
                                             EXP, scale=SCALE)
                    for c in range(nch):
                        qs = q0 + c * 512
                        o = off if c == 0 else 0
                        if kt == 0:
                            nc.vector.tensor_copy(acc[:, qs:qs + 512], ets[c][:])
                        else:
                            nc.vector.tensor_add(acc[:, qs + o:qs + 512],
                                                 acc[:, qs + o:qs + 512],
                                                 ets[c][:, o:512])
                    if fillers and kt >= 7:
                        fillers.pop(0)()
                    prev = (kt, off, ets)
                emit_av(*prev)
                finalize_a(3)
                finalize_b(3)
                while fillers:
                    fillers.pop(0)()

            def load_lhs(b, pool, tag, eng):
                # k-subtile order hh*8+i <-> global head 2i+hh (wout is
                # permuted host-side to match); one queue per head-pair
                lhs = pool.tile([128, KSUB, SCW], bf, tag=tag,
                                name=f"lhs{b}")
                for hh in range(HPC):
                    eng[hh % len(eng)].dma_start(
                        lhs[:, hh * NCORES:(hh + 1) * NCORES, :],
                        obs[(b, hh)][:].rearrange("i p s -> p i s"))
                return lhs

            def outproj(b, lhs, wos):
                for n in range(4):
                    wo = wos[n]
                    for m in range(SCW // 128):
                        ps = psA.tile([128, 512], f32, tag="bank")
                        for k in range(KSUB):
                            nc.tensor.matmul(
                                ps[:],
                                lhs[:, k, m * 128:(m + 1) * 128],
                                wo[:, k],
                                start=(k == 0), stop=(k == KSUB - 1))
                        ys = ep.tile([128, 512], f32, tag="ysT", name="ys")
                        nc.vector.tensor_copy(ys[:], ps[:])
                        nc.sync.dma_start(
                            y.ap()[b, m * 128:(m + 1) * 128,
                                   n * 512:(n + 1) * 512],
                            ys[:])

            def a2a(b, h):
                nc.gpsimd.collective_compute(
                    "AllToAll", mybir.AluOpType.bypass,
                    replica_groups=[list(range(NCORES))],
                    ins=[ibs[(b, h)].opt()], outs=[obs[(b, h)].opt()])

            # batch 0 compute; its A2A runs while batch 1 computes; all of
            # batch-0's outproj runs before batch-1's so the last A2A hides
            # under batch-0 outproj matmuls.
            wos = {}

            def prefetch_wo(n, pool, tag, eng):
                wo = pool.tile([128, KSUB, 512], bf, tag=tag, name=f"wo{n}")
                eng.dma_start(wo[:], wout.ap()[:, n])
                wos[n] = wo

            def load_consts():
                # rope/mask constants aren't needed until ~100us in;
                # emitting them after the chunk loop keeps the scalar queue
                # clear for x chunk slices during startup.
                nc.scalar.dma_start(cos_sb[:], cosg.ap())
                nc.scalar.dma_start(sin_sb[:], sing.ap())
                nc.scalar.dma_start(mneg_sb[:], mneg.ap())

            with nc.named_scope("qkv0"):
                qkT, Vn = qkv_rope(0, pre_xc=xc0, mid_hook=load_consts)
            with nc.named_scope("attn00"):
                attention(0, 0, qkT, Vn)
            a2a(0, 0)
            # prefetch batch-1 chunk-0 x while attention(0,1) computes
            xc1 = xp.tile([128, KSUB, SC], bf, tag="xc", name="xc")
            nc.scalar.dma_start(xc1[:], xT.ap()[:, NQC])
            # w_out chunks 2,3 live in their own pool; fetch early
            prefetch_wo(2, wop, "woA", nc.scalar)
            prefetch_wo(3, wop, "woB", nc.scalar)
            with nc.named_scope("attn01"):
                attention(0, 1, qkT, Vn)
            a2a(0, 1)
            with nc.named_scope("qkv1"):
                qkT, Vn = qkv_rope(1, pre_xc=xc1)
            # w_out chunks 0,1 reuse the (now idle) xc slots; gpsimd's DMA
            # queue is free after startup, and using it keeps the sync
            # queue clear for the attention ibs shipments.  lhs0 first: the
            # first outproj matmul needs it before the wo chunks.
            lhs0 = load_lhs(0, lp, "lhs0", [nc.gpsimd, nc.scalar])
            prefetch_wo(0, xp, "xc", nc.gpsimd)
            prefetch_wo(1, xp, "xc", nc.gpsimd)
            with nc.named_scope("attn10"):
                attention(1, 0, qkT, Vn)
            a2a(1, 0)
            with nc.named_scope("attn11"):
                attention(1, 1, qkT, Vn)
            a2a(1, 1)
            lhs1 = load_lhs(1, qp, "Vn", [nc.scalar, nc.gpsimd])
            with nc.named_scope("oproj0"):
                outproj(0, lhs0, wos)
            with nc.named_scope("oproj1"):
                outproj(1, lhs1, wos)

    nc.finalize()
    return nc


def _host_inputs(x, w_qkv, w_out):
    bf = ml_dtypes.bfloat16
    # x^T, chunk-major: [128, n_chunks, KSUB, SC] so one chunk is a single
    # contiguous 16KB line per partition.
    xTr = np.ascontiguousarray(
        x.reshape(B * S, D).T.reshape(KSUB, 128, B * S)
        .transpose(1, 0, 2).reshape(128, KSUB, B * S // SC, SC)
        .transpose(0, 2, 1, 3)
    ).astype(bf)
    horder = [2 * i + hh for hh in range(HPC) for i in range(NCORES)]
    woutr = np.ascontiguousarray(
        w_out.reshape(H, HD, D)[horder].transpose(1, 0, 2)
        .reshape(128, KSUB, 4, 512).transpose(0, 2, 1, 3)).astype(bf)

    half = HD // 2
    inv = (1.0 / (ROPE_BASE ** (np.arange(half, dtype=np.float32) / half))
           ).astype(np.float32)
    ang = (np.arange(S, dtype=np.float32)[:, None] * inv[None, :])  # [S, 64]
    c = np.cos(ang).astype(np.float32).T      # [64, S]
    s = np.sin(ang).astype(np.float32).T
    cosg = np.ascontiguousarray(np.concatenate([c, c], axis=0)).astype(bf)
    # pre-swapped: rows 0:64 = +sin (consumed against t[0:64] -> rt[64:128]),
    # rows 64:128 = -sin (consumed against t[64:128] -> rt[0:64])
    sing = np.ascontiguousarray(np.concatenate([s, -s], axis=0)).astype(bf)

    # mneg[p, j] = 0 where j >= p else -1e9 (upper-tri of the diagonal
    # 128-block).
    u = np.arange(128)[None, :]
    p = np.arange(128)[:, None]
    mneg = np.where(u >= p, 0.0, -1e9).astype(bf)

    maps = []
    for i in range(NCORES):
        h0, h1 = 2 * i, 2 * i + 1
        blocks = []
        for base in (0, D, 2 * D):
            blocks.append(w_qkv[:, base + 128 * h0:base + 128 * (h0 + 1)])
            blocks.append(w_qkv[:, base + 128 * h1:base + 128 * (h1 + 1)])
        shard = np.concatenate(blocks, axis=1)  # [D, 768]
        shard = shard.reshape(KSUB, 128, 3 * HPC * HD).transpose(1, 0, 2)
        # q/k tiles tile-major [128, 4, KSUB, 128]; v separate [128,KSUB,256]
        qk = np.ascontiguousarray(
            shard[:, :, :512].reshape(128, KSUB, 4, 128)
            .transpose(0, 2, 1, 3)).astype(bf)
        wv = np.ascontiguousarray(shard[:, :, 512:]).astype(bf)
        maps.append({"xT": xTr, "wqkv": qk, "wvg": wv, "wout": woutr,
                     "cosg": cosg, "sing": sing, "mneg": mneg})
    return maps


def kernel(x, w_qkv, w_out):
    from concourse.bass_utils import run_bass_kernel_spmd

    x = np.asarray(x, dtype=np.float32)
    w_qkv = np.asarray(w_qkv, dtype=np.float32)
    w_out = np.asarray(w_out, dtype=np.float32)

    if "nc" not in _CACHE:
        _CACHE["nc"] = _build()
    nc = _CACHE["nc"]

    trace = bool(int(os.environ.get("KERNEL_TRACE", "0")))
    if trace:
        trace = _install_trace_shim()

    in_maps = _host_inputs(x, w_qkv, w_out)
    res = run_bass_kernel_spmd(nc, in_maps, core_ids=list(range(NCORES)),
                               trace=trace)
    _CACHE["last_result"] = res
    # y per core i: [B, 256, D] = output rows [b*2048 + i*256, +256)
    full = np.empty((B * S, D), dtype=np.float32)
    for i in range(NCORES):
        yi = res.results[i]["y"]
        for b in range(B):
            full[b * S + i * SCW: b * S + (i + 1) * SCW] = yi[b]
    return full.reshape(B, S, D)


# revision 67
# speedup vs baseline: 1.1241x; 1.0164x over previous
"""Trainium2 Bass kernel for causal multi-head attention with RoPE.

Problem: x[2,2048,2048] -> qkv proj -> RoPE(q,k) -> causal softmax attention
(16 heads, hd=128) -> out proj.  Sharding: tensor-parallel over heads
(2 heads/core x 8 cores); the output projection contraction is restored
with one AllToAll per batch (head-shards -> sequence-shards), overlapped
with the other batch's compute, so each core computes a disjoint
[2, 256, 2048] slice of the final output.

All matmul operands are bf16 (PSUM accumulation stays fp32): halves
LDWEIGHTS time (the fp32 weight-load was the PE cadence limiter), halves
HBM/DMA traffic and the A2A payload, and doubles DVE throughput for the
elementwise work.  Softmax skips the max-subtraction (scores are O(1) by
construction); the causal mask is accumulated into PSUM as a -1e9
constant via a PE identity-matmul; softmax denominators are
partition-reduced and broadcast back with tiny ones-matmuls on the PE.
"""

import os
import sys

if "/opt/trn_rl_repo" not in sys.path:
    sys.path.insert(0, "/opt/trn_rl_repo")

import numpy as np
import ml_dtypes

B, S, D = 2, 2048, 2048
H, HD = 16, 128
NCORES = 8
HPC = H // NCORES          # heads per core (2)
ROPE_BASE = 10000.0
SCALE = 1.0 / float(np.sqrt(HD))
SC = 512                   # QKV matmul free-dim chunk (s positions)
KSUB = D // 128            # 16 contraction subtiles
SCW = S // NCORES          # 256: per-core output cols per batch

_CACHE = {}


def _install_trace_shim():
    """Optionally register the axon NTFF profile hook (for test.py tracing)."""
    try:
        import types

        if "antenv.axon_hooks" in sys.modules:
            return True
        import antenv
        from trn_agent_boot.trn_boot import _ntff_profile_via_ctypes

        hook = _ntff_profile_via_ctypes("/opt/axon/libaxon_pjrt.so")
        mod = types.ModuleType("antenv.axon_hooks")
        _state = {"hook": hook}
        mod.get_axon_ntff_profile_hook = lambda: _state["hook"]
        mod.set_axon_ntff_profile_hook = lambda h: _state.__setitem__("hook", h)
        sys.modules["antenv.axon_hooks"] = mod
        antenv.axon_hooks = mod
        return True
    except Exception:
        return False


def _build():
    import concourse.bass as bass  # noqa: F401
    import concourse.bass_isa as bass_isa
    import concourse.mybir as mybir
    import concourse.tile as tile
    from concourse import bacc
    from concourse.masks import make_identity

    f32 = mybir.dt.float32
    f32r = mybir.dt.float32r
    bf = mybir.dt.bfloat16
    EXP = mybir.ActivationFunctionType.Exp
    COPY = mybir.ActivationFunctionType.Copy

    nc = bacc.Bacc("TRN2", target_bir_lowering=False, debug=False,
                   num_devices=NCORES)

    # chunk-major DRAM layouts: each chunk/tile is one contiguous 4-16KB
    # line per partition, so a single DMA runs at full ring bandwidth.
    xT = nc.dram_tensor("xT", [128, B * S // SC, KSUB, SC], bf,
                        kind="ExternalInput")
    wqkv = nc.dram_tensor("wqkv", [128, 2 * HPC, KSUB, 128], bf,
                          kind="ExternalInput")
    wvg = nc.dram_tensor("wvg", [128, KSUB, HPC * HD], bf,
                         kind="ExternalInput")
    wout = nc.dram_tensor("wout", [128, 4, KSUB, 512], bf,
                          kind="ExternalInput")
    cosg = nc.dram_tensor("cosg", [128, S], bf, kind="ExternalInput")
    sing = nc.dram_tensor("sing", [128, S], bf, kind="ExternalInput")
    mneg = nc.dram_tensor("mneg", [128, 128], bf, kind="ExternalInput")
    y = nc.dram_tensor("y", [B, SCW, D], f32, kind="ExternalOutput")

    NQC = S // SC          # qkv s-chunks per batch
    NKT = S // 128         # 16 key tiles
    VOFF = 2 * HPC * HD    # v block column offset in w_sb (512)

    with tile.TileContext(nc) as tc:
        with tc.tile_pool(name="const", bufs=1) as cp, \
             tc.tile_pool(name="stage", bufs=1) as stp, \
             tc.tile_pool(name="dram", bufs=1, space="DRAM") as dp, \
             tc.tile_pool(name="psA", bufs=4, space="PSUM") as psA, \
             tc.tile_pool(name="psOut", bufs=1, space="PSUM") as psO, \
             tc.tile_pool(name="w", bufs=1) as wp, \
             tc.tile_pool(name="xc", bufs=2) as xp, \
             tc.tile_pool(name="wo2", bufs=1) as wop, \
             tc.tile_pool(name="lhs0", bufs=1) as lp, \
             tc.tile_pool(name="qkv", bufs=1) as qp, \
             tc.tile_pool(name="attn", bufs=1) as ap_, \
             tc.tile_pool(name="rotp", bufs=1) as rp, \
             tc.tile_pool(name="small", bufs=4) as ep:

            cos_sb = cp.tile([128, S], bf, name="cos_sb")
            sin_sb = cp.tile([128, S], bf, name="sin_sb")
            mneg_sb = cp.tile([128, 128], bf, name="mneg_sb")
            ident = cp.tile([128, 128], f32, name="ident")
            identB = cp.tile([128, 128], bf, name="identB")
            onescB = cp.tile([128, 1], bf, name="onescB")
            onesr = cp.tile([1, 128], f32, name="onesr")
            onesrR = cp.tile([1, 128], f32r, name="onesrR")

            # startup loads spread across the 3 DMA-capable queues
            # (sync/scalar/gpsimd) so the first chunk's matmuls are never
            # single-queue bound.
            # The 16 DMA engines are one shared ~400GB/s pool, so what
            # matters is byte ORDER across queues: first weight tiles, then
            # all of chunk 0 split 3 ways, then the rest.
            engs = [nc.sync, nc.scalar, nc.gpsimd]
            wqk_t = []
            for m in range(3):
                wt = wp.tile([128, KSUB, 128], bf, tag=f"w{m}", name=f"w{m}")
                engs[m].dma_start(wt[:], wqkv.ap()[:, m])
                wqk_t.append(wt)
            xc0 = xp.tile([128, KSUB, SC], bf, tag="xc", name="xc")
            for e, (k0, k1) in enumerate(((0, 6), (6, 11), (11, 16))):
                engs[e].dma_start(xc0[:, k0:k1], xT.ap()[:, 0, k0:k1])
            wt = wp.tile([128, KSUB, 128], bf, tag="w3", name="w3")
            nc.sync.dma_start(wt[:], wqkv.ap()[:, 3])
            wqk_t.append(wt)
            wv_t = wp.tile([128, KSUB, HPC * HD], bf, tag="wv", name="wv")
            nc.gpsimd.dma_start(wv_t[:], wvg.ap())

            make_identity(nc, ident[:])
            nc.vector.tensor_copy(identB[:], ident[:])
            nc.vector.memset(onescB[:], 1.0)
            nc.vector.memset(onesr[:], 1.0)
            nc.vector.tensor_copy(onesrR[:], onesr[:])

            ibs = {(b, h): dp.tile([NCORES, 128, SCW], bf, name=f"ib{b}{h}")
                   for b in range(B) for h in range(HPC)}
            obs = {(b, h): dp.tile([NCORES, 128, SCW], bf, name=f"ob{b}{h}")
                   for b in range(B) for h in range(HPC)}

            def qkv_rope(b, pre_xc=None, mid_hook=None):
                qkT = qp.tile([128, 2 * HPC, S], bf, tag="qkT")
                Vn = qp.tile([128, NKT, HPC * HD], bf, tag="Vn")

                # RoPE, fused halves (sin grid stored pre-swapped):
                # rt[0:64] = t[64:128]*(-sin); rt[64:128] = t[0:64]*(+sin);
                # t *= cos; t += rt.  Emitted per-m inside the last chunk so
                # the vector engine isn't backlogged when attention starts.
                def rope(m):
                    rt = rp.tile([128, S], bf, tag="rot", name="rt")
                    nc.vector.tensor_mul(rt[0:64, :],
                                         qkT[64:128, m],
                                         sin_sb[64:128, :])
                    nc.vector.tensor_mul(rt[64:128, :],
                                         qkT[0:64, m],
                                         sin_sb[0:64, :])
                    nc.vector.tensor_mul(qkT[:, m], qkT[:, m], cos_sb[:])
                    nc.vector.tensor_add(qkT[:, m], qkT[:, m], rt[:])

                for sc in range(NQC):
                    if sc == 0 and pre_xc is not None:
                        xc = pre_xc
                    else:
                        xc = xp.tile([128, KSUB, SC], bf, tag="xc", name="xc")
                        eng = (nc.scalar if sc == 1 else
                               nc.gpsimd if sc == 3 else nc.sync) \
                            if b == 0 else nc.sync
                        eng.dma_start(xc[:], xT.ap()[:, b * NQC + sc])
                    if sc == NQC - 1 and mid_hook is not None:
                        mid_hook()
                    for m in range(2 * HPC):
                        ps = psA.tile([128, 512], f32, tag="bank")
                        for k in range(KSUB):
                            nc.tensor.matmul(
                                ps[:, :SC],
                                wqk_t[m][:, k],
                                xc[:, k],
                                start=(k == 0), stop=(k == KSUB - 1))
                        nc.vector.tensor_copy(
                            qkT[:, m, sc * SC:(sc + 1) * SC], ps[:, :SC])
                        if sc == NQC - 1:
                            rope(m)
                    for st2 in range(SC // 128):
                        ps = psA.tile([128, 512], f32, tag="bank")
                        for k in range(KSUB):
                            nc.tensor.matmul(
                                ps[:, :HPC * HD],
                                xc[:, k, st2 * 128:(st2 + 1) * 128],
                                wv_t[:, k],
                                start=(k == 0), stop=(k == KSUB - 1))
                        nc.vector.tensor_copy(
                            Vn[:, sc * (SC // 128) + st2], ps[:, :HPC * HD])

                return qkT, Vn

            def attention(b, h, qkT, Vn, fillers=()):
                fillers = list(fillers)
                outT = psO.tile([128, S], f32, tag="outT")
                acc = ap_.tile([128, S], bf, tag="acc")

                def emit_av(kt, off, ets):
                    q0 = 512 * (kt // 4)
                    for c in range(len(ets)):
                        qs = q0 + c * 512
                        o = off if c == 0 else 0
                        nc.tensor.matmul(
                            outT[:, qs + o:qs + 512],
                            Vn[:, kt, h * 128:(h + 1) * 128],
                            ets[c][:, o:512],
                            start=(kt == 0),
                            stop=(kt == 4 * (qs // 512) + 3))

                st = rp.tile([128, S], bf, tag="rot", name="st")

                srows = {}

                def finalize_a(j):
                    # denominator: partition-reduce via ones-matmul, then
                    # reciprocal on vector.  Emitted one kt before
                    # finalize_b so the PE's bp matmul never waits on the
                    # vector queue.
                    rps = psA.tile([128, 512], f32, tag="bank")
                    nc.tensor.matmul(rps[0:1, :], onescB[:],
                                     acc[:, j * 512:(j + 1) * 512],
                                     start=True, stop=True)
                    srow = stp.tile([1, 512], f32, tag="srow")
                    srowR = stp.tile([1, 512], f32r, tag="srowR")
                    nc.vector.reciprocal_approx_fast(srow[:], rps[0:1, :])
                    nc.vector.tensor_copy(srowR[:], srow[:])
                    srows[j] = srowR

                def finalize_b(j):
                    # K=1 broadcast matmul, normalize, ship.  The PSUM->bf16
                    # copy runs on the scalar engine (idle once the kt-loop
                    # exps are done) so the vector queue only carries the
                    # final multiply.
                    bp = psA.tile([128, 512], f32, tag="bank")
                    nc.tensor.matmul(bp[:], onesrR[:], srows[j][:],
                                     start=True, stop=True)
                    sl = slice(j * 512, (j + 1) * 512)
                    nc.scalar.activation(st[:, sl], outT[:, sl], COPY)
                    nc.vector.tensor_mul(st[:, sl], st[:, sl], bp[:])
                    for jj in (2 * j, 2 * j + 1):
                        nc.sync.dma_start(ibs[(b, h)][jj],
                                          st[:, jj * SCW:(jj + 1) * SCW])

                prev = None
                for kt in range(NKT):
                    q0 = 512 * (kt // 4)
                    off = 128 * (kt % 4)   # causal start within chunk 0
                    nch = (S - q0) // 512
                    sps = []
                    for c in range(nch):
                        sp = psA.tile([128, 512], f32, tag="bank")
                        sps.append(sp)
                    # -1e9 upper-tri mask for the diagonal 128 block only;
                    # the rest of chunk 0 is a separate accumulation group.
                    nc.tensor.matmul(sps[0][:, off:off + 128], identB[:],
                                     mneg_sb[:], start=True, stop=False)
                    kT = qkT[:, HPC + h, kt * 128:(kt + 1) * 128]
                    nc.tensor.matmul(
                        sps[0][:, off:off + 128], kT,
                        qkT[:, h, q0 + off:q0 + off + 128],
                        start=False, stop=True)
                    if off + 128 < 512:
                        nc.tensor.matmul(
                            sps[0][:, off + 128:512], kT,
                            qkT[:, h, q0 + off + 128:q0 + 512],
                            start=True, stop=True)
                    for c in range(1, nch):
                        qs = q0 + c * 512
                        nc.tensor.matmul(
                            sps[c][:], kT,
                            qkT[:, h, qs:qs + 512],
                            start=True, stop=True)
                    if prev is not None:
                        emit_av(*prev)
                    if kt >= 5 and (kt - 5) % 4 == 0 and kt < 17:
                        finalize_a((kt - 5) // 4)
                    if kt >= 7 and (kt - 7) % 4 == 0:
                        finalize_b((kt - 7) // 4)
                    ets = []
                    for c in range(nch):
                        o = off if c == 0 else 0
                        et = ep.tile([128, 512], bf, tag="expT", bufs=6)
                        ets.append(et)
                        nc.scalar.activation(et[:, o:512], sps[c][:, o:512],
                                             EXP, scale=SCALE)
                    for c in range(nch):
                        qs = q0 + c * 512
                        o = off if c == 0 else 0
                        if kt == 0:
                            nc.vector.tensor_copy(acc[:, qs:qs + 512], ets[c][:])
                        else:
                            nc.vector.tensor_add(acc[:, qs + o:qs + 512],
                                                 acc[:, qs + o:qs + 512],
                                                 ets[c][:, o:512])
                    if fillers and kt >= 7:
                        fillers.pop(0)()
                    prev = (kt, off, ets)
                emit_av(*prev)
                finalize_a(3)
                finalize_b(3)
                while fillers:
                    fillers.pop(0)()

            def load_lhs(b, pool, tag, eng):
                # k-subtile order hh*8+i <-> global head 2i+hh (wout is
                # permuted host-side to match); one queue per head-pair
                lhs = pool.tile([128, KSUB, SCW], bf, tag=tag,
                                name=f"lhs{b}")
                for hh in range(HPC):
                    eng[hh % len(eng)].dma_start(
                        lhs[:, hh * NCORES:(hh + 1) * NCORES, :],
                        obs[(b, hh)][:].rearrange("i p s -> p i s"))
                return lhs

            def outproj(b, lhs, wos):
                for n in range(4):
                    wo = wos[n]
                    for m in range(SCW // 128):
                        ps = psA.tile([128, 512], f32, tag="bank")
                        for k in range(KSUB):
                            nc.tensor.matmul(
                                ps[:],
                                lhs[:, k, m * 128:(m + 1) * 128],
                                wo[:, k],
                                start=(k == 0), stop=(k == KSUB - 1))
                        ys = ep.tile([128, 512], f32, tag="ysT", name="ys")
                        nc.vector.tensor_copy(ys[:], ps[:])
                        nc.sync.dma_start(
                            y.ap()[b, m * 128:(m + 1) * 128,
                                   n * 512:(n + 1) * 512],
                            ys[:])

            def a2a(b, h):
                nc.gpsimd.collective_compute(
                    "AllToAll", mybir.AluOpType.bypass,
                    replica_groups=[list(range(NCORES))],
                    ins=[ibs[(b, h)].opt()], outs=[obs[(b, h)].opt()])

            # batch 0 compute; its A2A runs while batch 1 computes; all of
            # batch-0's outproj runs before batch-1's so the last A2A hides
            # under batch-0 outproj matmuls.
            wos = {}

            def prefetch_wo(n, pool, tag, eng):
                wo = pool.tile([128, KSUB, 512], bf, tag=tag, name=f"wo{n}")
                eng.dma_start(wo[:], wout.ap()[:, n])
                wos[n] = wo

            def load_consts():
                # rope/mask constants aren't needed until ~100us in;
                # emitting them after the chunk loop keeps the scalar queue
                # clear for x chunk slices during startup.
                nc.scalar.dma_start(cos_sb[:], cosg.ap())
                nc.scalar.dma_start(sin_sb[:], sing.ap())
                nc.scalar.dma_start(mneg_sb[:], mneg.ap())

            with nc.named_scope("qkv0"):
                qkT, Vn = qkv_rope(0, pre_xc=xc0, mid_hook=load_consts)
            with nc.named_scope("attn00"):
                attention(0, 0, qkT, Vn)
            a2a(0, 0)
            # prefetch batch-1 chunk-0 x while attention(0,1) computes
            xc1 = xp.tile([128, KSUB, SC], bf, tag="xc", name="xc")
            nc.scalar.dma_start(xc1[:], xT.ap()[:, NQC])
            # w_out chunks 2,3 live in their own pool; fetch early
            prefetch_wo(2, wop, "woA", nc.scalar)
            prefetch_wo(3, wop, "woB", nc.scalar)
            with nc.named_scope("attn01"):
                attention(0, 1, qkT, Vn)
            a2a(0, 1)
            with nc.named_scope("qkv1"):
                qkT, Vn = qkv_rope(1, pre_xc=xc1)
            # w_out chunks 0,1 reuse the (now idle) xc slots; gpsimd's DMA
            # queue is free after startup, and using it keeps the sync
            # queue clear for the attention ibs shipments.  lhs0 first: the
            # first outproj matmul needs it before the wo chunks.
            lhs0 = load_lhs(0, lp, "lhs0", [nc.gpsimd, nc.scalar])
            prefetch_wo(0, xp, "xc", nc.gpsimd)
            prefetch_wo(1, xp, "xc", nc.gpsimd)
            with nc.named_scope("attn10"):
                attention(1, 0, qkT, Vn)
            a2a(1, 0)
            with nc.named_scope("attn11"):
                attention(1, 1, qkT, Vn)
            a2a(1, 1)
            lhs1 = load_lhs(1, lp, "lhs1", [nc.scalar, nc.gpsimd])
            with nc.named_scope("oproj0"):
                outproj(0, lhs0, wos)
            with nc.named_scope("oproj1"):
                outproj(1, lhs1, wos)

    nc.finalize()
    return nc


def _host_inputs(x, w_qkv, w_out):
    bf = ml_dtypes.bfloat16
    # x^T, chunk-major: [128, n_chunks, KSUB, SC] so one chunk is a single
    # contiguous 16KB line per partition.
    xTr = np.ascontiguousarray(
        x.reshape(B * S, D).T.reshape(KSUB, 128, B * S)
        .transpose(1, 0, 2).reshape(128, KSUB, B * S // SC, SC)
        .transpose(0, 2, 1, 3)
    ).astype(bf)
    horder = [2 * i + hh for hh in range(HPC) for i in range(NCORES)]
    woutr = np.ascontiguousarray(
        w_out.reshape(H, HD, D)[horder].transpose(1, 0, 2)
        .reshape(128, KSUB, 4, 512).transpose(0, 2, 1, 3)).astype(bf)

    half = HD // 2
    inv = (1.0 / (ROPE_BASE ** (np.arange(half, dtype=np.float32) / half))
           ).astype(np.float32)
    ang = (np.arange(S, dtype=np.float32)[:, None] * inv[None, :])  # [S, 64]
    c = np.cos(ang).astype(np.float32).T      # [64, S]
    s = np.sin(ang).astype(np.float32).T
    cosg = np.ascontiguousarray(np.concatenate([c, c], axis=0)).astype(bf)
    # pre-swapped: rows 0:64 = +sin (consumed against t[0:64] -> rt[64:128]),
    # rows 64:128 = -sin (consumed against t[64:128] -> rt[0:64])
    sing = np.ascontiguousarray(np.concatenate([s, -s], axis=0)).astype(bf)

    # mneg[p, j] = 0 where j >= p else -1e9 (upper-tri of the diagonal
    # 128-block).
    u = np.arange(128)[None, :]
    p = np.arange(128)[:, None]
    mneg = np.where(u >= p, 0.0, -1e9).astype(bf)

    maps = []
    for i in range(NCORES):
        h0, h1 = 2 * i, 2 * i + 1
        blocks = []
        for base in (0, D, 2 * D):
            blocks.append(w_qkv[:, base + 128 * h0:base + 128 * (h0 + 1)])
            blocks.append(w_qkv[:, base + 128 * h1:base + 128 * (h1 + 1)])
        shard = np.concatenate(blocks, axis=1)  # [D, 768]
        shard = shard.reshape(KSUB, 128, 3 * HPC * HD).transpose(1, 0, 2)
        # q/k tiles tile-major [128, 4, KSUB, 128]; v separate [128,KSUB,256]
        qk = np.ascontiguousarray(
            shard[:, :, :512].reshape(128, KSUB, 4, 128)
            .transpose(0, 2, 1, 3)).astype(bf)
        wv = np.ascontiguousarray(shard[:, :, 512:]).astype(bf)
        maps.append({"xT": xTr, "wqkv": qk, "wvg": wv, "wout": woutr,
                     "cosg": cosg, "sing": sing, "mneg": mneg})
    return maps


def kernel(x, w_qkv, w_out):
    from concourse.bass_utils import run_bass_kernel_spmd

    x = np.asarray(x, dtype=np.float32)
    w_qkv = np.asarray(w_qkv, dtype=np.float32)
    w_out = np.asarray(w_out, dtype=np.float32)

    if "nc" not in _CACHE:
        _CACHE["nc"] = _build()
    nc = _CACHE["nc"]

    trace = bool(int(os.environ.get("KERNEL_TRACE", "0")))
    if trace:
        trace = _install_trace_shim()

    in_maps = _host_inputs(x, w_qkv, w_out)
    res = run_bass_kernel_spmd(nc, in_maps, core_ids=list(range(NCORES)),
                               trace=trace)
    _CACHE["last_result"] = res
    # y per core i: [B, 256, D] = output rows [b*2048 + i*256, +256)
    full = np.empty((B * S, D), dtype=np.float32)
    for i in range(NCORES):
        yi = res.results[i]["y"]
        for b in range(B):
            full[b * S + i * SCW: b * S + (i + 1) * SCW] = yi[b]
    return full.reshape(B, S, D)
